# revision 7
# baseline (speedup 1.0000x reference)
"""MoE layer (8 experts, top-2, D=1024, H=2048, N=4096 tokens) on 8 Trainium2
NeuronCores, via two SPMD launches:

Launch A — gate, token-parallel: each core takes 512 of the 4096 tokens
(x^T slice, feature-on-partition) and computes fp32 gate logits, the top-2
softmax weights (written as the routing matrix tok_w^T [8, 512]), and partial
sums for the aux load-balance loss. fp32 logits make the top-k routing match
the fp32 reference exactly.

Host dispatch: concatenate the per-core routing matrices to [8, 4096], build
each expert's token list, gather/pad the bf16 x columns per expert
(capacity 1152; overflow handled by extra FFN rounds), and finish the aux
scalar from the partial sums.

Launch B — FFN, expert-parallel: core e runs expert e over its gathered
tokens: h = gelu(x @ w1[e]) scaled by the gathered gate weights, y = h @ w2[e].
Matmuls are bf16 with fp32 PSUM accumulation; weights stay resident in SBUF
(loaded once per launch). Host scatter-adds the 8 outputs back to token order.

Self-contained: hardcodes the problem shapes; inputs are the full unsharded
tensors from setup_inputs(); returns (out [2,2048,1024] fp32, aux_loss fp32).
"""
import numpy as np
from contextlib import ExitStack

import ml_dtypes

import concourse.bass as bass
import concourse.tile as tile
from concourse import bacc, mybir
from concourse.alu_op_type import AluOpType
from concourse.bass_utils import run_bass_kernel_spmd
from concourse.masks import make_identity

DT = mybir.dt
AF = mybir.ActivationFunctionType
AX = mybir.AxisListType

B, S, D, H = 2, 2048, 1024, 2048
E, TOPK = 8, 2
AUX_COEFF = 0.01
NCORES = 8
NTOK = B * S            # 4096
TPC = NTOK // NCORES    # 512 tokens/core in the gate launch
DT_TILES = D // 128     # 8
HT_TILES = H // 128     # 16
TT_TILES = TPC // 128   # 4

CAP = 1152              # token capacity per expert per FFN round
NCH = 3                 # free-dim chunks per matmul group (PSUM bank = 512 fp32)
CH = CAP // NCH         # 384

_GATE_NC = None
_FFN_NC = None


def build_gate_nc():
    nc = bacc.Bacc("TRN2", target_bir_lowering=False, debug=False,
                   num_devices=NCORES)
    xt = nc.dram_tensor("xt", [D, TPC], DT.float32, kind="ExternalInput").ap()
    # gw pre-tiled on host to [128, DT_TILES, E] for a contiguous DMA
    gw = nc.dram_tensor("gw", [128, DT_TILES, E], DT.float32,
                        kind="ExternalInput").ap()
    gb = nc.dram_tensor("gb", [128, E], DT.float32, kind="ExternalInput").ap()
    wtf_out = nc.dram_tensor("wtf", [E, TPC], DT.float32, kind="ExternalOutput").ap()
    stats = nc.dram_tensor("stats", [1, 2], DT.float32, kind="ExternalOutput").ap()

    xt_r = xt.rearrange("(dt p) t -> p dt t", p=128)

    with tile.TileContext(nc) as tc, ExitStack() as ctx:
        persist = ctx.enter_context(tc.tile_pool(name="persist", bufs=1))
        xs_pool = ctx.enter_context(tc.tile_pool(name="xs", bufs=4))
        small = ctx.enter_context(tc.tile_pool(name="small", bufs=2))
        ps_g = ctx.enter_context(tc.tile_pool(name="psg", bufs=4, space="PSUM"))
        ps_m = ctx.enter_context(tc.tile_pool(name="psm", bufs=1, space="PSUM"))

        gw_sb = persist.tile([128, DT_TILES, E], DT.float32)
        gb_sb = persist.tile([128, E], DT.float32)
        logits_all = persist.tile([128, TT_TILES, E], DT.float32)
        tokw = persist.tile([128, TT_TILES, E], DT.float32)
        wtf = persist.tile([E, TPC], DT.float32)
        ident = persist.tile([128, 128], DT.float32)
        ones = persist.tile([128, 1], DT.float32)

        make_identity(nc, ident)
        nc.vector.memset(ones, 1.0)
        nc.sync.dma_start(out=gw_sb, in_=gw)
        nc.sync.dma_start(out=gb_sb, in_=gb)

        # logits in [token-partition, E-free] layout, fp32 exact
        for tt in range(TT_TILES):
            g_ps = ps_g.tile([128, E], DT.float32, tag="gps")
            xs = xs_pool.tile([128, DT_TILES, 128], DT.float32, tag="xs")
            nc.sync.dma_start(out=xs, in_=xt_r[:, :, tt * 128:(tt + 1) * 128])
            for dt in range(DT_TILES):
                nc.tensor.matmul(g_ps[:], xs[:, dt, :], gw_sb[:, dt, :],
                                 start=(dt == 0), stop=(dt == DT_TILES - 1))
            nc.vector.tensor_add(logits_all[:, tt, :], g_ps[:], gb_sb)

        # ---- batched gating math over all token tiles: [128, TT, E] ----
        T = TT_TILES

        def b3(t2):  # [128, T] -> broadcast [128, T, E]
            return t2.unsqueeze(2).broadcast_to((128, T, E))

        m1 = small.tile([128, T], DT.float32, tag="m1")
        nc.vector.tensor_reduce(m1, logits_all, AX.X, AluOpType.max)
        shifted = small.tile([128, T, E], DT.float32, tag="shifted")
        nc.vector.tensor_sub(shifted, logits_all, b3(m1))
        et = small.tile([128, T, E], DT.float32, tag="et")
        nc.scalar.activation(et, shifted, AF.Exp)
        sumexp = small.tile([128, T], DT.float32, tag="sumexp")
        nc.vector.tensor_reduce(sumexp, et, AX.X, AluOpType.add)
        lse_all = small.tile([128, T], DT.float32, tag="lse_all")
        ln_se = small.tile([128, T], DT.float32, tag="ln_se")
        nc.scalar.activation(ln_se, sumexp, AF.Ln)
        nc.vector.tensor_add(lse_all, ln_se, m1)
        s1_all = small.tile([128, T], DT.float32, tag="s1_all")
        nc.vector.tensor_reduce(s1_all, logits_all, AX.X, AluOpType.add)

        eq1 = small.tile([128, T, E], DT.float32, tag="eq1")
        nc.vector.tensor_scalar(out=eq1, in0=shifted, scalar1=0.0,
                                scalar2=None, op0=AluOpType.is_equal)
        masked = small.tile([128, T, E], DT.float32, tag="masked")
        nc.vector.scalar_tensor_tensor(out=masked, in0=eq1, scalar=-1e30,
                                       in1=shifted, op0=AluOpType.mult,
                                       op1=AluOpType.add)
        m2s = small.tile([128, T], DT.float32, tag="m2s")  # m2 - m1
        nc.vector.tensor_reduce(m2s, masked, AX.X, AluOpType.max)
        eq2 = small.tile([128, T, E], DT.float32, tag="eq2")
        nc.vector.tensor_tensor(eq2, masked, b3(m2s), AluOpType.is_equal)
        ed = small.tile([128, T], DT.float32, tag="ed")
        nc.scalar.activation(ed, m2s, AF.Exp)
        den = small.tile([128, T], DT.float32, tag="den")
        nc.vector.tensor_scalar_add(den, ed, 1.0)
        p1 = small.tile([128, T], DT.float32, tag="p1")
        nc.vector.reciprocal(p1, den)
        p2 = small.tile([128, T], DT.float32, tag="p2")
        nc.vector.tensor_mul(p2, ed, p1)
        t1 = small.tile([128, T, E], DT.float32, tag="t1")
        nc.vector.tensor_tensor(t1, eq1, b3(p1), AluOpType.mult)
        t2 = small.tile([128, T, E], DT.float32, tag="t2")
        nc.vector.tensor_tensor(t2, eq2, b3(p2), AluOpType.mult)
        nc.vector.tensor_add(tokw, t1, t2)

        # ---- aux-loss partial sums: S1 = sum(logits), S2 = sum(lse) ----
        s_cat = small.tile([128, 2], DT.float32, tag="s_cat")
        nc.vector.tensor_reduce(s_cat[:, 0:1], s1_all, AX.X, AluOpType.add)
        nc.vector.tensor_reduce(s_cat[:, 1:2], lse_all, AX.X, AluOpType.add)
        st_ps = ps_m.tile([1, 2], DT.float32, tag="misc")
        nc.tensor.matmul(st_ps[:], ones[:], s_cat, start=True, stop=True)
        st_sb = small.tile([1, 2], DT.float32, tag="st_sb")
        nc.vector.tensor_copy(st_sb, st_ps[:])
        nc.sync.dma_start(out=stats, in_=st_sb)

        # tok_w -> [E, tokens] routing matrix
        for tt in range(TT_TILES):
            tr_ps = ps_m.tile([E, 128], DT.float32, tag="misc")
            nc.tensor.transpose(tr_ps[:], tokw[:, tt, :], ident)
            nc.vector.tensor_copy(wtf[:, tt * 128:(tt + 1) * 128], tr_ps[:])
        nc.sync.dma_start(out=wtf_out, in_=wtf)

    nc.compile()
    return nc


def build_ffn_nc():
    nc = bacc.Bacc("TRN2", target_bir_lowering=False, debug=False,
                   num_devices=NCORES)
    xg = nc.dram_tensor("xg", [D, CAP], DT.bfloat16, kind="ExternalInput").ap()
    wg = nc.dram_tensor("wg", [1, CAP], DT.float32, kind="ExternalInput").ap()
    w1 = nc.dram_tensor("w1", [D, H], DT.bfloat16, kind="ExternalInput").ap()
    w2 = nc.dram_tensor("w2", [H, D], DT.bfloat16, kind="ExternalInput").ap()
    yg = nc.dram_tensor("yg", [D, CAP], DT.float32, kind="ExternalOutput").ap()

    xg_r = xg.rearrange("(dt p) t -> p dt t", p=128)
    w1_r = w1.rearrange("(dt p) h -> p dt h", p=128)
    w2_r = w2.rearrange("(ht p) d -> p ht d", p=128)
    yg_r = yg.rearrange("(dt p) t -> p dt t", p=128)

    with tile.TileContext(nc) as tc, ExitStack() as ctx:
        persist = ctx.enter_context(tc.tile_pool(name="persist", bufs=1))
        ps_h = ctx.enter_context(tc.tile_pool(name="psh", bufs=3, space="PSUM"))
        ps_y = ctx.enter_context(tc.tile_pool(name="psy", bufs=3, space="PSUM"))

        xg_sb = persist.tile([128, DT_TILES, CAP], DT.bfloat16)
        wb = persist.tile([128, CAP], DT.bfloat16)
        wbf = persist.tile([128, CAP], DT.float32)
        w1_sb = persist.tile([128, DT_TILES, H], DT.bfloat16)
        w2_sb = persist.tile([128, HT_TILES, D], DT.bfloat16)
        h_sb = persist.tile([128, HT_TILES, CAP], DT.bfloat16)
        y_sb = persist.tile([128, DT_TILES, CAP], DT.float32)

        # input DMAs ordered/chunked so the first matmuls' deps land first
        nc.sync.dma_start(out=w1_sb[:, :, 0:H // 8], in_=w1_r[:, :, 0:H // 8])
        for dt in range(DT_TILES):
            nc.sync.dma_start(out=xg_sb[:, dt, 0:CH], in_=xg_r[:, dt, 0:CH])
        for ch in range(1, NCH):
            cs = slice(ch * CH, (ch + 1) * CH)
            nc.sync.dma_start(out=xg_sb[:, :, cs], in_=xg_r[:, :, cs])
        nc.gpsimd.dma_start(out=wbf, in_=wg[0:1, :].to_broadcast((128, CAP)))
        nc.vector.tensor_copy(wb, wbf)
        for q in range(1, 8):
            hs = slice(q * (H // 8), (q + 1) * (H // 8))
            nc.sync.dma_start(out=w1_sb[:, :, hs], in_=w1_r[:, :, hs])
        for q in range(4):
            ts_ = slice(q * (HT_TILES // 4), (q + 1) * (HT_TILES // 4))
            nc.sync.dma_start(out=w2_sb[:, ts_, :], in_=w2_r[:, ts_, :])

        # layer 1: h^T[ht, tok] = gelu(w1^T x^T), scaled by gate weights
        for ht in range(HT_TILES):
            for ch in range(NCH):
                cs = slice(ch * CH, (ch + 1) * CH)
                h_ps = ps_h.tile([128, CH], DT.float32, tag="hps")
                for dt in range(DT_TILES):
                    nc.tensor.matmul(h_ps[:],
                                     w1_sb[:, dt, ht * 128:(ht + 1) * 128],
                                     xg_sb[:, dt, cs],
                                     start=(dt == 0),
                                     stop=(dt == DT_TILES - 1))
                nc.scalar.activation(h_sb[:, ht, cs], h_ps[:], AF.Gelu)
                nc.vector.tensor_mul(h_sb[:, ht, cs], h_sb[:, ht, cs],
                                     wb[:, cs])
        # layer 2: y^T[dt, tok] = w2^T h^T
        for dt in range(DT_TILES):
            for ch in range(NCH):
                cs = slice(ch * CH, (ch + 1) * CH)
                y_ps = ps_y.tile([128, CH], DT.float32, tag="yps")
                for ht in range(HT_TILES):
                    nc.tensor.matmul(y_ps[:],
                                     w2_sb[:, ht, dt * 128:(dt + 1) * 128],
                                     h_sb[:, ht, cs],
                                     start=(ht == 0),
                                     stop=(ht == HT_TILES - 1))
                nc.vector.tensor_copy(y_sb[:, dt, cs], y_ps[:])
            # stream this d-tile out while later tiles compute
            nc.sync.dma_start(out=yg_r[:, dt, :], in_=y_sb[:, dt, :])

    nc.compile()
    return nc


def _get_gate_nc():
    global _GATE_NC
    if _GATE_NC is None:
        _GATE_NC = build_gate_nc()
    return _GATE_NC


def _get_ffn_nc():
    global _FFN_NC
    if _FFN_NC is None:
        _FFN_NC = build_ffn_nc()
    return _FFN_NC


def run_gate(xt_full, gate_w, gate_b):
    gb_b = np.ascontiguousarray(np.broadcast_to(gate_b, (128, E))).astype(np.float32)
    gw_pre = np.ascontiguousarray(
        gate_w.reshape(DT_TILES, 128, E).transpose(1, 0, 2))
    in_maps = [{
        "xt": np.ascontiguousarray(xt_full[:, c * TPC:(c + 1) * TPC]),
        "gw": gw_pre,
        "gb": gb_b,
    } for c in range(NCORES)]
    res = run_bass_kernel_spmd(_get_gate_nc(), in_maps,
                               core_ids=list(range(NCORES)))
    wtf = np.concatenate([res.results[c]["wtf"] for c in range(NCORES)],
                         axis=1)                       # [E, NTOK]
    s1 = float(sum(res.results[c]["stats"][0, 0] for c in range(NCORES)))
    s2 = float(sum(res.results[c]["stats"][0, 1] for c in range(NCORES)))
    return wtf, s1, s2


def run_ffn_round(xbT, wtf, idx_lists, w1b, w2b):
    """One expert-parallel round over up to CAP tokens per expert."""
    in_maps = []
    for e in range(NCORES):
        idx = idx_lists[e]
        n = len(idx)
        pad_idx = np.zeros(CAP, dtype=np.int64)
        pad_idx[:n] = idx
        xg = np.ascontiguousarray(xbT[:, pad_idx])     # [D, CAP] bf16
        wg = np.zeros((1, CAP), dtype=np.float32)
        wg[0, :n] = wtf[e, idx]
        in_maps.append({
            "xg": xg,
            "wg": wg,
            "w1": w1b[e],
            "w2": w2b[e],
        })
    res = run_bass_kernel_spmd(_get_ffn_nc(), in_maps,
                               core_ids=list(range(NCORES)))
    return [res.results[e]["yg"] for e in range(NCORES)]


def kernel(x, gate_w, gate_b, w1, w2):
    x = np.asarray(x, dtype=np.float32)
    gate_w = np.asarray(gate_w, dtype=np.float32)
    gate_b = np.asarray(gate_b, dtype=np.float32)
    assert x.shape == (B, S, D), x.shape

    x_flat = x.reshape(NTOK, D)
    xt_full = np.ascontiguousarray(x_flat.T)                      # [D, NTOK] f32
    xbT = xt_full.astype(ml_dtypes.bfloat16)                      # [D, NTOK] bf16
    w1b = np.asarray(w1).astype(ml_dtypes.bfloat16)
    w2b = np.asarray(w2).astype(ml_dtypes.bfloat16)

    # ---- launch A: gate ----
    wtf, s1, s2 = run_gate(xt_full, gate_w, gate_b)

    # ---- host dispatch + launch B rounds + unshard (scatter-add) ----
    full_lists = [np.nonzero(wtf[e])[0] for e in range(E)]

    out_t = np.zeros((NTOK, D), dtype=np.float32)
    offset = 0
    while True:
        round_lists = [fl[offset:offset + CAP] for fl in full_lists]
        if all(len(r) == 0 for r in round_lists):
            break
        ygs = run_ffn_round(xbT, wtf, round_lists, w1b, w2b)
        for e in range(E):
            idx = round_lists[e]
            if len(idx):
                out_t[idx] += ygs[e][:, :len(idx)].T
        offset += CAP

    out = out_t.reshape(B, S, D)

    t = 1.0 / E
    aux = AUX_COEFF * (t * np.log(t) - t * (s1 - E * s2) / (NTOK * E))
    return out, np.float32(aux)


# revision 15
# speedup vs baseline: 1.0302x; 1.0302x over previous
"""MoE layer (8 experts, top-2, D=1024, H=2048, N=4096 tokens) on 8 Trainium2
NeuronCores, via two SPMD launches:

Launch A — gate, token-parallel: each core takes 512 of the 4096 tokens
(x^T slice, feature-on-partition) and computes fp32 gate logits, the top-2
softmax weights (written as the routing matrix tok_w^T [8, 512]), and partial
sums for the aux load-balance loss. fp32 logits make the top-k routing match
the fp32 reference exactly.

Host dispatch: concatenate the per-core routing matrices to [8, 4096], build
each expert's token list, gather/pad the bf16 x columns per expert
(capacity 1152; overflow handled by extra FFN rounds), and finish the aux
scalar from the partial sums.

Launch B — FFN, expert-parallel: core e runs expert e over its gathered
tokens: h = gelu(x @ w1[e]) scaled by the gathered gate weights, y = h @ w2[e].
Matmuls are bf16 with fp32 PSUM accumulation; weights stay resident in SBUF
(loaded once per launch). Host scatter-adds the 8 outputs back to token order.

Self-contained: hardcodes the problem shapes; inputs are the full unsharded
tensors from setup_inputs(); returns (out [2,2048,1024] fp32, aux_loss fp32).
"""
import numpy as np
from contextlib import ExitStack

import ml_dtypes

import concourse.bass as bass
import concourse.tile as tile
from concourse import bacc, mybir
from concourse.alu_op_type import AluOpType
from concourse.bass_utils import run_bass_kernel_spmd

DT = mybir.dt
AF = mybir.ActivationFunctionType
AX = mybir.AxisListType

B, S, D, H = 2, 2048, 1024, 2048
E, TOPK = 8, 2
AUX_COEFF = 0.01
NCORES = 8
NTOK = B * S            # 4096
TPC = NTOK // NCORES    # 512 tokens/core in the gate launch
DT_TILES = D // 128     # 8
HT_TILES = H // 128     # 16
TT_TILES = TPC // 128   # 4

CAP = 1122              # token capacity per expert per FFN round
NCH = 3                 # free-dim chunks per matmul group (PSUM bank = 512 fp32)
CH = CAP // NCH         # 374

_GATE_NC = None
_FFN_NC = None


def build_gate_nc():
    nc = bacc.Bacc("TRN2", target_bir_lowering=False, debug=False,
                   num_devices=NCORES)
    xt = nc.dram_tensor("xt", [D, TPC], DT.float32, kind="ExternalInput").ap()
    # gw pre-tiled on host to [128, DT_TILES, E] for a contiguous DMA
    gw = nc.dram_tensor("gw", [128, DT_TILES, E], DT.float32,
                        kind="ExternalInput").ap()
    gb = nc.dram_tensor("gb", [128, E], DT.float32, kind="ExternalInput").ap()
    idin = nc.dram_tensor("idin", [128, 128], DT.float32,
                          kind="ExternalInput").ap()
    wtf_out = nc.dram_tensor("wtf", [E, TPC], DT.float32, kind="ExternalOutput").ap()
    stats = nc.dram_tensor("stats", [1, 2], DT.float32, kind="ExternalOutput").ap()

    xt_r = xt.rearrange("(dt p) t -> p dt t", p=128)

    with tile.TileContext(nc) as tc, ExitStack() as ctx:
        persist = ctx.enter_context(tc.tile_pool(name="persist", bufs=1))
        small = ctx.enter_context(tc.tile_pool(name="small", bufs=2))
        ps_g = ctx.enter_context(tc.tile_pool(name="psg", bufs=2, space="PSUM"))
        ps_m = ctx.enter_context(tc.tile_pool(name="psm", bufs=2, space="PSUM"))

        gw_sb = persist.tile([128, DT_TILES, E], DT.float32)
        gb_sb = persist.tile([128, E], DT.float32)
        xs = persist.tile([128, DT_TILES, TPC], DT.float32)
        lsb = persist.tile([E, TPC], DT.float32)
        logits_all = persist.tile([128, TT_TILES, E], DT.float32)
        tokw = persist.tile([128, TT_TILES, E], DT.float32)
        wtf = persist.tile([E, TPC], DT.float32)
        ident = persist.tile([128, 128], DT.float32)
        ones = persist.tile([128, 1], DT.float32)

        nc.vector.memset(ones, 1.0)
        nc.sync.dma_start(out=gw_sb, in_=gw)
        nc.sync.dma_start(out=gb_sb, in_=gb)
        nc.sync.dma_start(out=ident, in_=idin)
        for dt in range(DT_TILES):
            nc.sync.dma_start(out=xs[:, dt, :], in_=xt_r[:, dt, :])

        # logits^T [E, tok] with gw stationary (tiny weight loads),
        # x moving at N=512 fp32
        lg_ps = ps_g.tile([E, TPC], DT.float32, tag="gps")
        for dt in range(DT_TILES):
            nc.tensor.matmul(lg_ps[:], gw_sb[:, dt, :], xs[:, dt, :],
                             start=(dt == 0), stop=(dt == DT_TILES - 1))
        nc.vector.tensor_copy(lsb, lg_ps[:])
        # transpose back to [token-partition, E-free] tiles and add bias
        for tt in range(TT_TILES):
            tr_ps = ps_m.tile([128, E], DT.float32, tag="ltr")
            nc.tensor.transpose(tr_ps[:], lsb[:, tt * 128:(tt + 1) * 128],
                                ident[0:E, 0:E])
            nc.vector.tensor_add(logits_all[:, tt, :], tr_ps[:], gb_sb)

        # ---- batched gating math over all token tiles: [128, TT, E] ----
        T = TT_TILES

        def b3(t2):  # [128, T] -> broadcast [128, T, E]
            return t2.unsqueeze(2).broadcast_to((128, T, E))

        m1 = small.tile([128, T], DT.float32, tag="m1")
        nc.vector.tensor_reduce(m1, logits_all, AX.X, AluOpType.max)
        shifted = small.tile([128, T, E], DT.float32, tag="shifted")
        nc.vector.tensor_sub(shifted, logits_all, b3(m1))
        et = small.tile([128, T, E], DT.float32, tag="et")
        nc.scalar.activation(et, shifted, AF.Exp)
        sumexp = small.tile([128, T], DT.float32, tag="sumexp")
        nc.vector.tensor_reduce(sumexp, et, AX.X, AluOpType.add)
        lse_all = small.tile([128, T], DT.float32, tag="lse_all")
        ln_se = small.tile([128, T], DT.float32, tag="ln_se")
        nc.scalar.activation(ln_se, sumexp, AF.Ln)
        nc.vector.tensor_add(lse_all, ln_se, m1)
        s1_all = small.tile([128, T], DT.float32, tag="s1_all")
        nc.vector.tensor_reduce(s1_all, logits_all, AX.X, AluOpType.add)

        eq1 = small.tile([128, T, E], DT.float32, tag="eq1")
        nc.vector.tensor_scalar(out=eq1, in0=shifted, scalar1=0.0,
                                scalar2=None, op0=AluOpType.is_equal)
        masked = small.tile([128, T, E], DT.float32, tag="masked")
        nc.vector.scalar_tensor_tensor(out=masked, in0=eq1, scalar=-1e30,
                                       in1=shifted, op0=AluOpType.mult,
                                       op1=AluOpType.add)
        m2s = small.tile([128, T], DT.float32, tag="m2s")  # m2 - m1
        nc.vector.tensor_reduce(m2s, masked, AX.X, AluOpType.max)
        eq2 = small.tile([128, T, E], DT.float32, tag="eq2")
        nc.vector.tensor_tensor(eq2, masked, b3(m2s), AluOpType.is_equal)
        ed = small.tile([128, T], DT.float32, tag="ed")
        nc.scalar.activation(ed, m2s, AF.Exp)
        den = small.tile([128, T], DT.float32, tag="den")
        nc.vector.tensor_scalar_add(den, ed, 1.0)
        p1 = small.tile([128, T], DT.float32, tag="p1")
        nc.vector.reciprocal(p1, den)
        p2 = small.tile([128, T], DT.float32, tag="p2")
        nc.vector.tensor_mul(p2, ed, p1)
        t1 = small.tile([128, T, E], DT.float32, tag="t1")
        nc.vector.tensor_tensor(t1, eq1, b3(p1), AluOpType.mult)
        t2 = small.tile([128, T, E], DT.float32, tag="t2")
        nc.vector.tensor_tensor(t2, eq2, b3(p2), AluOpType.mult)
        nc.vector.tensor_add(tokw, t1, t2)

        # ---- aux-loss partial sums: S1 = sum(logits), S2 = sum(lse) ----
        s_cat = small.tile([128, 2], DT.float32, tag="s_cat")
        nc.vector.tensor_reduce(s_cat[:, 0:1], s1_all, AX.X, AluOpType.add)
        nc.vector.tensor_reduce(s_cat[:, 1:2], lse_all, AX.X, AluOpType.add)
        st_ps = ps_m.tile([1, 2], DT.float32, tag="misc")
        nc.tensor.matmul(st_ps[:], ones[:], s_cat, start=True, stop=True)
        st_sb = small.tile([1, 2], DT.float32, tag="st_sb")
        nc.vector.tensor_copy(st_sb, st_ps[:])
        nc.sync.dma_start(out=stats, in_=st_sb)

        # tok_w -> [E, tokens] routing matrix
        for tt in range(TT_TILES):
            tr_ps = ps_m.tile([E, 128], DT.float32, tag="misc")
            nc.tensor.transpose(tr_ps[:], tokw[:, tt, :], ident)
            nc.vector.tensor_copy(wtf[:, tt * 128:(tt + 1) * 128], tr_ps[:])
        nc.sync.dma_start(out=wtf_out, in_=wtf)

    nc.compile()
    return nc


def build_ffn_nc():
    nc = bacc.Bacc("TRN2", target_bir_lowering=False, debug=False,
                   num_devices=NCORES)
    xg = nc.dram_tensor("xg", [D, CAP], DT.bfloat16, kind="ExternalInput").ap()
    # gate weights pre-broadcast to 128 partitions on the host
    wgb = nc.dram_tensor("wgb", [128, CAP], DT.bfloat16, kind="ExternalInput").ap()
    w1 = nc.dram_tensor("w1", [D, H], DT.bfloat16, kind="ExternalInput").ap()
    w2 = nc.dram_tensor("w2", [H, D], DT.bfloat16, kind="ExternalInput").ap()
    yg = nc.dram_tensor("yg", [D, CAP], DT.float32, kind="ExternalOutput").ap()

    xg_r = xg.rearrange("(dt p) t -> p dt t", p=128)
    w1_r = w1.rearrange("(dt p) h -> p dt h", p=128)
    w2_r = w2.rearrange("(ht p) d -> p ht d", p=128)
    yg_r = yg.rearrange("(dt p) t -> p dt t", p=128)

    with tile.TileContext(nc) as tc, ExitStack() as ctx:
        persist = ctx.enter_context(tc.tile_pool(name="persist", bufs=1))
        ps_h = ctx.enter_context(tc.tile_pool(name="psh", bufs=3, space="PSUM"))
        ps_y = ctx.enter_context(tc.tile_pool(name="psy", bufs=3, space="PSUM"))

        xg_sb = persist.tile([128, DT_TILES, CAP], DT.bfloat16)
        wb = persist.tile([128, CAP], DT.bfloat16)
        w1_sb = persist.tile([128, DT_TILES, H], DT.bfloat16)
        w2_sb = persist.tile([128, HT_TILES, D], DT.bfloat16)
        h_sb = persist.tile([128, HT_TILES, CAP], DT.bfloat16)
        y_sb = persist.tile([128, DT_TILES, CAP], DT.float32)

        # input DMAs ordered/chunked so the first matmuls' deps land first
        nc.sync.dma_start(out=w1_sb[:, :, 0:H // 8], in_=w1_r[:, :, 0:H // 8])
        for dt in range(DT_TILES):
            nc.sync.dma_start(out=xg_sb[:, dt, 0:CH], in_=xg_r[:, dt, 0:CH])
        nc.sync.dma_start(out=wb, in_=wgb)
        nc.sync.dma_start(out=w1_sb[:, :, H // 8:2 * (H // 8)],
                          in_=w1_r[:, :, H // 8:2 * (H // 8)])
        nc.sync.dma_start(out=xg_sb[:, :, CH:2 * CH], in_=xg_r[:, :, CH:2 * CH])
        nc.sync.dma_start(out=w1_sb[:, :, 2 * (H // 8):3 * (H // 8)],
                          in_=w1_r[:, :, 2 * (H // 8):3 * (H // 8)])
        nc.sync.dma_start(out=xg_sb[:, :, 2 * CH:3 * CH],
                          in_=xg_r[:, :, 2 * CH:3 * CH])
        for q in range(3, 8):
            hs = slice(q * (H // 8), (q + 1) * (H // 8))
            nc.sync.dma_start(out=w1_sb[:, :, hs], in_=w1_r[:, :, hs])
        for q in range(4):
            ts_ = slice(q * (HT_TILES // 4), (q + 1) * (HT_TILES // 4))
            nc.sync.dma_start(out=w2_sb[:, ts_, :], in_=w2_r[:, ts_, :])

        # layer 1: h^T[ht, tok] = gelu(w1^T x^T), scaled by gate weights
        for ht in range(HT_TILES):
            for ch in range(NCH):
                cs = slice(ch * CH, (ch + 1) * CH)
                h_ps = ps_h.tile([128, CH], DT.float32, tag="hps")
                for dt in range(DT_TILES):
                    nc.tensor.matmul(h_ps[:],
                                     w1_sb[:, dt, ht * 128:(ht + 1) * 128],
                                     xg_sb[:, dt, cs],
                                     start=(dt == 0),
                                     stop=(dt == DT_TILES - 1))
                nc.scalar.activation(h_sb[:, ht, cs], h_ps[:], AF.Gelu)
                nc.vector.tensor_mul(h_sb[:, ht, cs], h_sb[:, ht, cs],
                                     wb[:, cs])
        # layer 2: y^T[dt, tok] = w2^T h^T
        for dt in range(DT_TILES):
            for ch in range(NCH):
                cs = slice(ch * CH, (ch + 1) * CH)
                y_ps = ps_y.tile([128, CH], DT.float32, tag="yps")
                for ht in range(HT_TILES):
                    nc.tensor.matmul(y_ps[:],
                                     w2_sb[:, ht, dt * 128:(dt + 1) * 128],
                                     h_sb[:, ht, cs],
                                     start=(ht == 0),
                                     stop=(ht == HT_TILES - 1))
                nc.vector.tensor_copy(y_sb[:, dt, cs], y_ps[:])
            # stream this d-tile out while later tiles compute
            nc.sync.dma_start(out=yg_r[:, dt, :], in_=y_sb[:, dt, :])

    nc.compile()
    return nc


def _get_gate_nc():
    global _GATE_NC
    if _GATE_NC is None:
        _GATE_NC = build_gate_nc()
    return _GATE_NC


def _get_ffn_nc():
    global _FFN_NC
    if _FFN_NC is None:
        _FFN_NC = build_ffn_nc()
    return _FFN_NC


def run_gate(xt_full, gate_w, gate_b):
    gb_b = np.ascontiguousarray(np.broadcast_to(gate_b, (128, E))).astype(np.float32)
    gw_pre = np.ascontiguousarray(
        gate_w.reshape(DT_TILES, 128, E).transpose(1, 0, 2))
    ident = np.eye(128, dtype=np.float32)
    in_maps = [{
        "xt": np.ascontiguousarray(xt_full[:, c * TPC:(c + 1) * TPC]),
        "gw": gw_pre,
        "gb": gb_b,
        "idin": ident,
    } for c in range(NCORES)]
    res = run_bass_kernel_spmd(_get_gate_nc(), in_maps,
                               core_ids=list(range(NCORES)))
    wtf = np.concatenate([res.results[c]["wtf"] for c in range(NCORES)],
                         axis=1)                       # [E, NTOK]
    s1 = float(sum(res.results[c]["stats"][0, 0] for c in range(NCORES)))
    s2 = float(sum(res.results[c]["stats"][0, 1] for c in range(NCORES)))
    return wtf, s1, s2


def run_ffn_round(xbT, wtf, idx_lists, w1b, w2b):
    """One expert-parallel round over up to CAP tokens per expert."""
    in_maps = []
    for e in range(NCORES):
        idx = idx_lists[e]
        n = len(idx)
        pad_idx = np.zeros(CAP, dtype=np.int64)
        pad_idx[:n] = idx
        xg = np.ascontiguousarray(xbT[:, pad_idx])     # [D, CAP] bf16
        wg = np.zeros(CAP, dtype=np.float32)
        wg[:n] = wtf[e, idx]
        wgb = np.ascontiguousarray(
            np.broadcast_to(wg.astype(ml_dtypes.bfloat16), (128, CAP)))
        in_maps.append({
            "xg": xg,
            "wgb": wgb,
            "w1": w1b[e],
            "w2": w2b[e],
        })
    res = run_bass_kernel_spmd(_get_ffn_nc(), in_maps,
                               core_ids=list(range(NCORES)))
    return [res.results[e]["yg"] for e in range(NCORES)]


def kernel(x, gate_w, gate_b, w1, w2):
    x = np.asarray(x, dtype=np.float32)
    gate_w = np.asarray(gate_w, dtype=np.float32)
    gate_b = np.asarray(gate_b, dtype=np.float32)
    assert x.shape == (B, S, D), x.shape

    x_flat = x.reshape(NTOK, D)
    xt_full = np.ascontiguousarray(x_flat.T)                      # [D, NTOK] f32
    xbT = xt_full.astype(ml_dtypes.bfloat16)                      # [D, NTOK] bf16
    w1b = np.asarray(w1).astype(ml_dtypes.bfloat16)
    w2b = np.asarray(w2).astype(ml_dtypes.bfloat16)

    # ---- launch A: gate ----
    wtf, s1, s2 = run_gate(xt_full, gate_w, gate_b)

    # ---- host dispatch + launch B rounds + unshard (scatter-add) ----
    full_lists = [np.nonzero(wtf[e])[0] for e in range(E)]

    out_t = np.zeros((NTOK, D), dtype=np.float32)
    offset = 0
    while True:
        round_lists = [fl[offset:offset + CAP] for fl in full_lists]
        if all(len(r) == 0 for r in round_lists):
            break
        ygs = run_ffn_round(xbT, wtf, round_lists, w1b, w2b)
        for e in range(E):
            idx = round_lists[e]
            if len(idx):
                out_t[idx] += ygs[e][:, :len(idx)].T
        offset += CAP

    out = out_t.reshape(B, S, D)

    t = 1.0 / E
    aux = AUX_COEFF * (t * np.log(t) - t * (s1 - E * s2) / (NTOK * E))
    return out, np.float32(aux)


# revision 17
# speedup vs baseline: 1.0470x; 1.0163x over previous
"""MoE layer (8 experts, top-2, D=1024, H=2048, N=4096 tokens) on 8 Trainium2
NeuronCores, via two SPMD launches:

Launch A — gate, token-parallel: each core takes 512 of the 4096 tokens
(x^T slice, feature-on-partition) and computes fp32 gate logits, the top-2
softmax weights (written as the routing matrix tok_w^T [8, 512]), and partial
sums for the aux load-balance loss. fp32 logits make the top-k routing match
the fp32 reference exactly.

Host dispatch: concatenate the per-core routing matrices to [8, 4096], build
each expert's token list, gather/pad the bf16 x columns per expert
(capacity 1152; overflow handled by extra FFN rounds), and finish the aux
scalar from the partial sums.

Launch B — FFN, expert-parallel: core e runs expert e over its gathered
tokens: h = gelu(x @ w1[e]) scaled by the gathered gate weights, y = h @ w2[e].
Matmuls are bf16 with fp32 PSUM accumulation; weights stay resident in SBUF
(loaded once per launch). Host scatter-adds the 8 outputs back to token order.

Self-contained: hardcodes the problem shapes; inputs are the full unsharded
tensors from setup_inputs(); returns (out [2,2048,1024] fp32, aux_loss fp32).
"""
import numpy as np
from contextlib import ExitStack

import ml_dtypes

import concourse.bass as bass
import concourse.tile as tile
from concourse import bacc, mybir
from concourse.alu_op_type import AluOpType
from concourse.bass_utils import run_bass_kernel_spmd

DT = mybir.dt
AF = mybir.ActivationFunctionType
AX = mybir.AxisListType

B, S, D, H = 2, 2048, 1024, 2048
E, TOPK = 8, 2
AUX_COEFF = 0.01
NCORES = 8
NTOK = B * S            # 4096
TPC = NTOK // NCORES    # 512 tokens/core in the gate launch
DT_TILES = D // 128     # 8
HT_TILES = H // 128     # 16
TT_TILES = TPC // 128   # 4

CAP = 1120              # token capacity per expert per FFN round
# free-dim chunks per matmul group; 512 = one fp32 PSUM bank, 16B-aligned bf16
CHUNKS = [(0, 512), (512, 512), (1024, 96)]

_GATE_NC = None
_FFN_NC = None


def build_gate_nc():
    nc = bacc.Bacc("TRN2", target_bir_lowering=False, debug=False,
                   num_devices=NCORES)
    xt = nc.dram_tensor("xt", [D, TPC], DT.float32, kind="ExternalInput").ap()
    # gw pre-tiled on host to [128, DT_TILES, E] for a contiguous DMA
    gw = nc.dram_tensor("gw", [128, DT_TILES, E], DT.float32,
                        kind="ExternalInput").ap()
    gb = nc.dram_tensor("gb", [128, E], DT.float32, kind="ExternalInput").ap()
    idin = nc.dram_tensor("idin", [128, 128], DT.float32,
                          kind="ExternalInput").ap()
    wtf_out = nc.dram_tensor("wtf", [E, TPC], DT.float32, kind="ExternalOutput").ap()
    stats = nc.dram_tensor("stats", [1, 2], DT.float32, kind="ExternalOutput").ap()

    xt_r = xt.rearrange("(dt p) t -> p dt t", p=128)

    with tile.TileContext(nc) as tc, ExitStack() as ctx:
        persist = ctx.enter_context(tc.tile_pool(name="persist", bufs=1))
        small = ctx.enter_context(tc.tile_pool(name="small", bufs=2))
        ps_g = ctx.enter_context(tc.tile_pool(name="psg", bufs=2, space="PSUM"))
        ps_m = ctx.enter_context(tc.tile_pool(name="psm", bufs=2, space="PSUM"))

        gw_sb = persist.tile([128, DT_TILES, E], DT.float32)
        gb_sb = persist.tile([128, E], DT.float32)
        xs = persist.tile([128, DT_TILES, TPC], DT.float32)
        lsb = persist.tile([E, TPC], DT.float32)
        logits_all = persist.tile([128, TT_TILES, E], DT.float32)
        tokw = persist.tile([128, TT_TILES, E], DT.float32)
        wtf = persist.tile([E, TPC], DT.float32)
        ident = persist.tile([128, 128], DT.float32)
        ones = persist.tile([128, 1], DT.float32)

        nc.vector.memset(ones, 1.0)
        nc.sync.dma_start(out=gw_sb, in_=gw)
        nc.sync.dma_start(out=gb_sb, in_=gb)
        nc.sync.dma_start(out=ident, in_=idin)
        for dt in range(DT_TILES):
            nc.sync.dma_start(out=xs[:, dt, :], in_=xt_r[:, dt, :])

        # logits^T [E, tok] with gw stationary (tiny weight loads),
        # x moving at N=512 fp32
        lg_ps = ps_g.tile([E, TPC], DT.float32, tag="gps")
        for dt in range(DT_TILES):
            nc.tensor.matmul(lg_ps[:], gw_sb[:, dt, :], xs[:, dt, :],
                             start=(dt == 0), stop=(dt == DT_TILES - 1))
        nc.vector.tensor_copy(lsb, lg_ps[:])
        # transpose back to [token-partition, E-free] tiles and add bias
        for tt in range(TT_TILES):
            tr_ps = ps_m.tile([128, E], DT.float32, tag="ltr")
            nc.tensor.transpose(tr_ps[:], lsb[:, tt * 128:(tt + 1) * 128],
                                ident[0:E, 0:E])
            nc.vector.tensor_add(logits_all[:, tt, :], tr_ps[:], gb_sb)

        # ---- batched gating math over all token tiles: [128, TT, E] ----
        T = TT_TILES

        def b3(t2):  # [128, T] -> broadcast [128, T, E]
            return t2.unsqueeze(2).broadcast_to((128, T, E))

        m1 = small.tile([128, T], DT.float32, tag="m1")
        nc.vector.tensor_reduce(m1, logits_all, AX.X, AluOpType.max)
        shifted = small.tile([128, T, E], DT.float32, tag="shifted")
        nc.vector.tensor_sub(shifted, logits_all, b3(m1))
        et = small.tile([128, T, E], DT.float32, tag="et")
        nc.scalar.activation(et, shifted, AF.Exp)
        sumexp = small.tile([128, T], DT.float32, tag="sumexp")
        nc.vector.tensor_reduce(sumexp, et, AX.X, AluOpType.add)
        lse_all = small.tile([128, T], DT.float32, tag="lse_all")
        ln_se = small.tile([128, T], DT.float32, tag="ln_se")
        nc.scalar.activation(ln_se, sumexp, AF.Ln)
        nc.vector.tensor_add(lse_all, ln_se, m1)
        s1_all = small.tile([128, T], DT.float32, tag="s1_all")
        nc.vector.tensor_reduce(s1_all, logits_all, AX.X, AluOpType.add)

        eq1 = small.tile([128, T, E], DT.float32, tag="eq1")
        nc.vector.tensor_scalar(out=eq1, in0=shifted, scalar1=0.0,
                                scalar2=None, op0=AluOpType.is_equal)
        masked = small.tile([128, T, E], DT.float32, tag="masked")
        nc.vector.scalar_tensor_tensor(out=masked, in0=eq1, scalar=-1e30,
                                       in1=shifted, op0=AluOpType.mult,
                                       op1=AluOpType.add)
        m2s = small.tile([128, T], DT.float32, tag="m2s")  # m2 - m1
        nc.vector.tensor_reduce(m2s, masked, AX.X, AluOpType.max)
        eq2 = small.tile([128, T, E], DT.float32, tag="eq2")
        nc.vector.tensor_tensor(eq2, masked, b3(m2s), AluOpType.is_equal)
        ed = small.tile([128, T], DT.float32, tag="ed")
        nc.scalar.activation(ed, m2s, AF.Exp)
        den = small.tile([128, T], DT.float32, tag="den")
        nc.vector.tensor_scalar_add(den, ed, 1.0)
        p1 = small.tile([128, T], DT.float32, tag="p1")
        nc.vector.reciprocal(p1, den)
        p2 = small.tile([128, T], DT.float32, tag="p2")
        nc.vector.tensor_mul(p2, ed, p1)
        t1 = small.tile([128, T, E], DT.float32, tag="t1")
        nc.vector.tensor_tensor(t1, eq1, b3(p1), AluOpType.mult)
        t2 = small.tile([128, T, E], DT.float32, tag="t2")
        nc.vector.tensor_tensor(t2, eq2, b3(p2), AluOpType.mult)
        nc.vector.tensor_add(tokw, t1, t2)

        # ---- aux-loss partial sums: S1 = sum(logits), S2 = sum(lse) ----
        s_cat = small.tile([128, 2], DT.float32, tag="s_cat")
        nc.vector.tensor_reduce(s_cat[:, 0:1], s1_all, AX.X, AluOpType.add)
        nc.vector.tensor_reduce(s_cat[:, 1:2], lse_all, AX.X, AluOpType.add)
        st_ps = ps_m.tile([1, 2], DT.float32, tag="misc")
        nc.tensor.matmul(st_ps[:], ones[:], s_cat, start=True, stop=True)
        st_sb = small.tile([1, 2], DT.float32, tag="st_sb")
        nc.vector.tensor_copy(st_sb, st_ps[:])
        nc.sync.dma_start(out=stats, in_=st_sb)

        # tok_w -> [E, tokens] routing matrix
        for tt in range(TT_TILES):
            tr_ps = ps_m.tile([E, 128], DT.float32, tag="misc")
            nc.tensor.transpose(tr_ps[:], tokw[:, tt, :], ident)
            nc.vector.tensor_copy(wtf[:, tt * 128:(tt + 1) * 128], tr_ps[:])
        nc.sync.dma_start(out=wtf_out, in_=wtf)

    nc.compile()
    return nc


def build_ffn_nc():
    nc = bacc.Bacc("TRN2", target_bir_lowering=False, debug=False,
                   num_devices=NCORES)
    xg = nc.dram_tensor("xg", [D, CAP], DT.bfloat16, kind="ExternalInput").ap()
    # gate weights pre-broadcast to 128 partitions on the host
    wgb = nc.dram_tensor("wgb", [128, CAP], DT.bfloat16, kind="ExternalInput").ap()
    w1 = nc.dram_tensor("w1", [D, H], DT.bfloat16, kind="ExternalInput").ap()
    w2 = nc.dram_tensor("w2", [H, D], DT.bfloat16, kind="ExternalInput").ap()
    yg = nc.dram_tensor("yg", [D, CAP], DT.float32, kind="ExternalOutput").ap()

    xg_r = xg.rearrange("(dt p) t -> p dt t", p=128)
    w1_r = w1.rearrange("(dt p) h -> p dt h", p=128)
    w2_r = w2.rearrange("(ht p) d -> p ht d", p=128)
    yg_r = yg.rearrange("(dt p) t -> p dt t", p=128)

    with tile.TileContext(nc) as tc, ExitStack() as ctx:
        persist = ctx.enter_context(tc.tile_pool(name="persist", bufs=1))
        ps_h = ctx.enter_context(tc.tile_pool(name="psh", bufs=3, space="PSUM"))
        ps_y = ctx.enter_context(tc.tile_pool(name="psy", bufs=3, space="PSUM"))

        xg_sb = persist.tile([128, DT_TILES, CAP], DT.bfloat16)
        wb = persist.tile([128, CAP], DT.bfloat16)
        w1_sb = persist.tile([128, DT_TILES, H], DT.bfloat16)
        w2_sb = persist.tile([128, HT_TILES, D], DT.bfloat16)
        h_sb = persist.tile([128, HT_TILES, CAP], DT.bfloat16)
        y_sb = persist.tile([128, DT_TILES, CAP], DT.float32)

        # input DMAs ordered/chunked so the first matmuls' deps land first
        nc.sync.dma_start(out=w1_sb[:, :, 0:H // 8], in_=w1_r[:, :, 0:H // 8])
        c0, w0 = CHUNKS[0]
        for dt in range(DT_TILES):
            nc.sync.dma_start(out=xg_sb[:, dt, c0:c0 + w0],
                              in_=xg_r[:, dt, c0:c0 + w0])
        for c, w in CHUNKS[1:]:
            nc.sync.dma_start(out=xg_sb[:, :, c:c + w], in_=xg_r[:, :, c:c + w])
        nc.sync.dma_start(out=wb, in_=wgb)
        for q in range(1, 8):
            hs = slice(q * (H // 8), (q + 1) * (H // 8))
            nc.sync.dma_start(out=w1_sb[:, :, hs], in_=w1_r[:, :, hs])
        for q in range(4):
            ts_ = slice(q * (HT_TILES // 4), (q + 1) * (HT_TILES // 4))
            nc.sync.dma_start(out=w2_sb[:, ts_, :], in_=w2_r[:, ts_, :])

        # layer 1: h^T[ht, tok] = gelu(w1^T x^T), scaled by gate weights
        for ht in range(HT_TILES):
            for c, w in CHUNKS:
                cs = slice(c, c + w)
                h_ps = ps_h.tile([128, w], DT.float32, tag="hps")
                for dt in range(DT_TILES):
                    nc.tensor.matmul(h_ps[:],
                                     w1_sb[:, dt, ht * 128:(ht + 1) * 128],
                                     xg_sb[:, dt, cs],
                                     start=(dt == 0),
                                     stop=(dt == DT_TILES - 1))
                nc.scalar.activation(h_sb[:, ht, cs], h_ps[:], AF.Gelu)
                nc.vector.tensor_mul(h_sb[:, ht, cs], h_sb[:, ht, cs],
                                     wb[:, cs])
        # layer 2: y^T[dt, tok] = w2^T h^T
        for dt in range(DT_TILES):
            for c, w in CHUNKS:
                cs = slice(c, c + w)
                y_ps = ps_y.tile([128, w], DT.float32, tag="yps")
                for ht in range(HT_TILES):
                    nc.tensor.matmul(y_ps[:],
                                     w2_sb[:, ht, dt * 128:(dt + 1) * 128],
                                     h_sb[:, ht, cs],
                                     start=(ht == 0),
                                     stop=(ht == HT_TILES - 1))
                nc.vector.tensor_copy(y_sb[:, dt, cs], y_ps[:])
            # stream this d-tile out while later tiles compute
            nc.sync.dma_start(out=yg_r[:, dt, :], in_=y_sb[:, dt, :])

    nc.compile()
    return nc


def _get_gate_nc():
    global _GATE_NC
    if _GATE_NC is None:
        _GATE_NC = build_gate_nc()
    return _GATE_NC


def _get_ffn_nc():
    global _FFN_NC
    if _FFN_NC is None:
        _FFN_NC = build_ffn_nc()
    return _FFN_NC


def run_gate(xt_full, gate_w, gate_b):
    gb_b = np.ascontiguousarray(np.broadcast_to(gate_b, (128, E))).astype(np.float32)
    gw_pre = np.ascontiguousarray(
        gate_w.reshape(DT_TILES, 128, E).transpose(1, 0, 2))
    ident = np.eye(128, dtype=np.float32)
    in_maps = [{
        "xt": np.ascontiguousarray(xt_full[:, c * TPC:(c + 1) * TPC]),
        "gw": gw_pre,
        "gb": gb_b,
        "idin": ident,
    } for c in range(NCORES)]
    res = run_bass_kernel_spmd(_get_gate_nc(), in_maps,
                               core_ids=list(range(NCORES)))
    wtf = np.concatenate([res.results[c]["wtf"] for c in range(NCORES)],
                         axis=1)                       # [E, NTOK]
    s1 = float(sum(res.results[c]["stats"][0, 0] for c in range(NCORES)))
    s2 = float(sum(res.results[c]["stats"][0, 1] for c in range(NCORES)))
    return wtf, s1, s2


def run_ffn_round(xbT, wtf, idx_lists, w1b, w2b):
    """One expert-parallel round over up to CAP tokens per expert."""
    in_maps = []
    for e in range(NCORES):
        idx = idx_lists[e]
        n = len(idx)
        pad_idx = np.zeros(CAP, dtype=np.int64)
        pad_idx[:n] = idx
        xg = np.ascontiguousarray(xbT[:, pad_idx])     # [D, CAP] bf16
        wg = np.zeros(CAP, dtype=np.float32)
        wg[:n] = wtf[e, idx]
        wgb = np.ascontiguousarray(
            np.broadcast_to(wg.astype(ml_dtypes.bfloat16), (128, CAP)))
        in_maps.append({
            "xg": xg,
            "wgb": wgb,
            "w1": w1b[e],
            "w2": w2b[e],
        })
    res = run_bass_kernel_spmd(_get_ffn_nc(), in_maps,
                               core_ids=list(range(NCORES)))
    return [res.results[e]["yg"] for e in range(NCORES)]


def kernel(x, gate_w, gate_b, w1, w2):
    x = np.asarray(x, dtype=np.float32)
    gate_w = np.asarray(gate_w, dtype=np.float32)
    gate_b = np.asarray(gate_b, dtype=np.float32)
    assert x.shape == (B, S, D), x.shape

    x_flat = x.reshape(NTOK, D)
    xt_full = np.ascontiguousarray(x_flat.T)                      # [D, NTOK] f32
    xbT = xt_full.astype(ml_dtypes.bfloat16)                      # [D, NTOK] bf16
    w1b = np.asarray(w1).astype(ml_dtypes.bfloat16)
    w2b = np.asarray(w2).astype(ml_dtypes.bfloat16)

    # ---- launch A: gate ----
    wtf, s1, s2 = run_gate(xt_full, gate_w, gate_b)

    # ---- host dispatch + launch B rounds + unshard (scatter-add) ----
    full_lists = [np.nonzero(wtf[e])[0] for e in range(E)]

    out_t = np.zeros((NTOK, D), dtype=np.float32)
    offset = 0
    while True:
        round_lists = [fl[offset:offset + CAP] for fl in full_lists]
        if all(len(r) == 0 for r in round_lists):
            break
        ygs = run_ffn_round(xbT, wtf, round_lists, w1b, w2b)
        for e in range(E):
            idx = round_lists[e]
            if len(idx):
                out_t[idx] += ygs[e][:, :len(idx)].T
        offset += CAP

    out = out_t.reshape(B, S, D)

    t = 1.0 / E
    aux = AUX_COEFF * (t * np.log(t) - t * (s1 - E * s2) / (NTOK * E))
    return out, np.float32(aux)


# revision 20
# speedup vs baseline: 1.0634x; 1.0157x over previous
"""MoE layer (8 experts, top-2, D=1024, H=2048, N=4096 tokens) on 8 Trainium2
NeuronCores, via two SPMD launches:

Launch A — gate, token-parallel: each core takes 512 of the 4096 tokens
(x^T slice, feature-on-partition) and computes fp32 gate logits, the top-2
softmax weights (written as the routing matrix tok_w^T [8, 512]), and partial
sums for the aux load-balance loss. fp32 logits make the top-k routing match
the fp32 reference exactly.

Host dispatch: concatenate the per-core routing matrices to [8, 4096], build
each expert's token list, gather/pad the bf16 x columns per expert
(capacity 1152; overflow handled by extra FFN rounds), and finish the aux
scalar from the partial sums.

Launch B — FFN, expert-parallel: core e runs expert e over its gathered
tokens: h = gelu(x @ w1[e]) scaled by the gathered gate weights, y = h @ w2[e].
Matmuls are bf16 with fp32 PSUM accumulation; weights stay resident in SBUF
(loaded once per launch). Host scatter-adds the 8 outputs back to token order.

Self-contained: hardcodes the problem shapes; inputs are the full unsharded
tensors from setup_inputs(); returns (out [2,2048,1024] fp32, aux_loss fp32).
"""
import numpy as np
from contextlib import ExitStack

import ml_dtypes

import concourse.bass as bass
import concourse.tile as tile
from concourse import bacc, mybir
from concourse.alu_op_type import AluOpType
from concourse.bass_utils import run_bass_kernel_spmd

DT = mybir.dt
AF = mybir.ActivationFunctionType
AX = mybir.AxisListType

B, S, D, H = 2, 2048, 1024, 2048
E, TOPK = 8, 2
AUX_COEFF = 0.01
NCORES = 8
NTOK = B * S            # 4096
TPC = NTOK // NCORES    # 512 tokens/core in the gate launch
DT_TILES = D // 128     # 8
HT_TILES = H // 128     # 16
TT_TILES = TPC // 128   # 4

CAP = 1120              # token capacity per expert per FFN round
# free-dim chunks per matmul group; 512 = one fp32 PSUM bank, 16B-aligned bf16
CHUNKS = [(0, 512), (512, 512), (1024, 96)]

_GATE_NC = None
_FFN_NC = None


def build_gate_nc():
    nc = bacc.Bacc("TRN2", target_bir_lowering=False, debug=False,
                   num_devices=NCORES)
    xt = nc.dram_tensor("xt", [D, TPC], DT.float32, kind="ExternalInput").ap()
    # gw pre-tiled on host to [128, DT_TILES, E] for a contiguous DMA
    gw = nc.dram_tensor("gw", [128, DT_TILES, E], DT.float32,
                        kind="ExternalInput").ap()
    gb = nc.dram_tensor("gb", [128, E], DT.float32, kind="ExternalInput").ap()
    idin = nc.dram_tensor("idin", [128, 128], DT.float32,
                          kind="ExternalInput").ap()
    wtf_out = nc.dram_tensor("wtf", [E, TPC], DT.float32, kind="ExternalOutput").ap()
    stats = nc.dram_tensor("stats", [1, 2], DT.float32, kind="ExternalOutput").ap()

    xt_r = xt.rearrange("(dt p) t -> p dt t", p=128)

    with tile.TileContext(nc) as tc, ExitStack() as ctx:
        persist = ctx.enter_context(tc.tile_pool(name="persist", bufs=1))
        small = ctx.enter_context(tc.tile_pool(name="small", bufs=2))
        ps_g = ctx.enter_context(tc.tile_pool(name="psg", bufs=2, space="PSUM"))
        ps_m = ctx.enter_context(tc.tile_pool(name="psm", bufs=2, space="PSUM"))

        gw_sb = persist.tile([128, DT_TILES, E], DT.float32)
        gb_sb = persist.tile([128, E], DT.float32)
        xs_t = [persist.tile([128, TPC], DT.float32, name=f"xs{dt}",
                             tag=f"xs{dt}")
                for dt in range(DT_TILES)]
        lsb = persist.tile([E, TPC], DT.float32)
        logits_all = persist.tile([128, TT_TILES, E], DT.float32)
        tokw = persist.tile([128, TT_TILES, E], DT.float32)
        wtf = persist.tile([E, TPC], DT.float32)
        ident = persist.tile([128, 128], DT.float32)
        ones = persist.tile([128, 1], DT.float32)

        nc.vector.memset(ones, 1.0)
        nc.sync.dma_start(out=gw_sb, in_=gw)
        for dt in range(DT_TILES):
            nc.sync.dma_start(out=xs_t[dt], in_=xt_r[:, dt, :])
        nc.sync.dma_start(out=gb_sb, in_=gb)
        nc.sync.dma_start(out=ident, in_=idin)

        # logits^T [E, tok] with gw stationary (tiny weight loads),
        # x moving at N=512 fp32
        lg_ps = ps_g.tile([E, TPC], DT.float32, tag="gps")
        for dt in range(DT_TILES):
            nc.tensor.matmul(lg_ps[:], gw_sb[:, dt, :], xs_t[dt][:],
                             start=(dt == 0), stop=(dt == DT_TILES - 1))
        nc.vector.tensor_copy(lsb, lg_ps[:])
        # transpose back to [token-partition, E-free] tiles and add bias
        for tt in range(TT_TILES):
            tr_ps = ps_m.tile([128, E], DT.float32, tag="ltr")
            nc.tensor.transpose(tr_ps[:], lsb[:, tt * 128:(tt + 1) * 128],
                                ident[0:E, 0:E])
            nc.vector.tensor_add(logits_all[:, tt, :], tr_ps[:], gb_sb)

        # ---- batched gating math over all token tiles: [128, TT, E] ----
        T = TT_TILES

        def b3(t2):  # [128, T] -> broadcast [128, T, E]
            return t2.unsqueeze(2).broadcast_to((128, T, E))

        m1 = small.tile([128, T], DT.float32, tag="m1")
        nc.vector.tensor_reduce(m1, logits_all, AX.X, AluOpType.max)
        shifted = small.tile([128, T, E], DT.float32, tag="shifted")
        nc.vector.tensor_sub(shifted, logits_all, b3(m1))
        et = small.tile([128, T, E], DT.float32, tag="et")
        nc.scalar.activation(et, shifted, AF.Exp)
        sumexp = small.tile([128, T], DT.float32, tag="sumexp")
        nc.vector.tensor_reduce(sumexp, et, AX.X, AluOpType.add)
        lse_all = small.tile([128, T], DT.float32, tag="lse_all")
        ln_se = small.tile([128, T], DT.float32, tag="ln_se")
        nc.scalar.activation(ln_se, sumexp, AF.Ln)
        nc.vector.tensor_add(lse_all, ln_se, m1)
        s1_all = small.tile([128, T], DT.float32, tag="s1_all")
        nc.vector.tensor_reduce(s1_all, logits_all, AX.X, AluOpType.add)

        eq1 = small.tile([128, T, E], DT.float32, tag="eq1")
        nc.vector.tensor_scalar(out=eq1, in0=shifted, scalar1=0.0,
                                scalar2=None, op0=AluOpType.is_equal)
        masked = small.tile([128, T, E], DT.float32, tag="masked")
        nc.vector.scalar_tensor_tensor(out=masked, in0=eq1, scalar=-1e30,
                                       in1=shifted, op0=AluOpType.mult,
                                       op1=AluOpType.add)
        m2s = small.tile([128, T], DT.float32, tag="m2s")  # m2 - m1
        nc.vector.tensor_reduce(m2s, masked, AX.X, AluOpType.max)
        eq2 = small.tile([128, T, E], DT.float32, tag="eq2")
        nc.vector.tensor_tensor(eq2, masked, b3(m2s), AluOpType.is_equal)
        ed = small.tile([128, T], DT.float32, tag="ed")
        nc.scalar.activation(ed, m2s, AF.Exp)
        den = small.tile([128, T], DT.float32, tag="den")
        nc.vector.tensor_scalar_add(den, ed, 1.0)
        p1 = small.tile([128, T], DT.float32, tag="p1")
        nc.vector.reciprocal(p1, den)
        p2 = small.tile([128, T], DT.float32, tag="p2")
        nc.vector.tensor_mul(p2, ed, p1)
        t1 = small.tile([128, T, E], DT.float32, tag="t1")
        nc.vector.tensor_tensor(t1, eq1, b3(p1), AluOpType.mult)
        t2 = small.tile([128, T, E], DT.float32, tag="t2")
        nc.vector.tensor_tensor(t2, eq2, b3(p2), AluOpType.mult)
        nc.vector.tensor_add(tokw, t1, t2)

        # ---- aux-loss partial sums: S1 = sum(logits), S2 = sum(lse) ----
        s_cat = small.tile([128, 2], DT.float32, tag="s_cat")
        nc.vector.tensor_reduce(s_cat[:, 0:1], s1_all, AX.X, AluOpType.add)
        nc.vector.tensor_reduce(s_cat[:, 1:2], lse_all, AX.X, AluOpType.add)
        st_ps = ps_m.tile([1, 2], DT.float32, tag="misc")
        nc.tensor.matmul(st_ps[:], ones[:], s_cat, start=True, stop=True)
        st_sb = small.tile([1, 2], DT.float32, tag="st_sb")
        nc.vector.tensor_copy(st_sb, st_ps[:])
        nc.sync.dma_start(out=stats, in_=st_sb)

        # tok_w -> [E, tokens] routing matrix
        for tt in range(TT_TILES):
            tr_ps = ps_m.tile([E, 128], DT.float32, tag="misc")
            nc.tensor.transpose(tr_ps[:], tokw[:, tt, :], ident)
            nc.vector.tensor_copy(wtf[:, tt * 128:(tt + 1) * 128], tr_ps[:])
        nc.sync.dma_start(out=wtf_out, in_=wtf)

    nc.compile()
    return nc


def build_ffn_nc():
    nc = bacc.Bacc("TRN2", target_bir_lowering=False, debug=False,
                   num_devices=NCORES)
    xg = nc.dram_tensor("xg", [D, CAP], DT.bfloat16, kind="ExternalInput").ap()
    # gate weights pre-broadcast to 128 partitions on the host
    wgb = nc.dram_tensor("wgb", [128, CAP], DT.bfloat16, kind="ExternalInput").ap()
    w1 = nc.dram_tensor("w1", [D, H], DT.bfloat16, kind="ExternalInput").ap()
    w2 = nc.dram_tensor("w2", [H, D], DT.bfloat16, kind="ExternalInput").ap()
    yg = nc.dram_tensor("yg", [D, CAP], DT.float32, kind="ExternalOutput").ap()

    xg_r = xg.rearrange("(dt p) t -> p dt t", p=128)
    w1_r = w1.rearrange("(dt p) h -> p dt h", p=128)
    w2_r = w2.rearrange("(ht p) d -> p ht d", p=128)
    yg_r = yg.rearrange("(dt p) t -> p dt t", p=128)

    with tile.TileContext(nc) as tc, ExitStack() as ctx:
        persist = ctx.enter_context(tc.tile_pool(name="persist", bufs=1))
        ps_h = ctx.enter_context(tc.tile_pool(name="psh", bufs=3, space="PSUM"))
        ps_y = ctx.enter_context(tc.tile_pool(name="psy", bufs=3, space="PSUM"))

        xg_sb = persist.tile([128, DT_TILES, CAP], DT.bfloat16)
        wb = persist.tile([128, CAP], DT.bfloat16)
        w1_sb = persist.tile([128, DT_TILES, H], DT.bfloat16)
        w2_sb = persist.tile([128, HT_TILES, D], DT.bfloat16)
        h_sb = persist.tile([128, HT_TILES, CAP], DT.bfloat16)
        y_sb = persist.tile([128, DT_TILES, CAP], DT.float32)

        # input DMAs ordered/chunked so the first matmuls' deps land first
        nc.sync.dma_start(out=w1_sb[:, :, 0:H // 8], in_=w1_r[:, :, 0:H // 8])
        c0, w0 = CHUNKS[0]
        for dt in range(DT_TILES):
            nc.sync.dma_start(out=xg_sb[:, dt, c0:c0 + w0],
                              in_=xg_r[:, dt, c0:c0 + w0])
        for c, w in CHUNKS[1:]:
            nc.sync.dma_start(out=xg_sb[:, :, c:c + w], in_=xg_r[:, :, c:c + w])
        nc.sync.dma_start(out=wb, in_=wgb)
        for q in range(1, 8):
            hs = slice(q * (H // 8), (q + 1) * (H // 8))
            nc.sync.dma_start(out=w1_sb[:, :, hs], in_=w1_r[:, :, hs])
        for q in range(4):
            ts_ = slice(q * (HT_TILES // 4), (q + 1) * (HT_TILES // 4))
            nc.sync.dma_start(out=w2_sb[:, ts_, :], in_=w2_r[:, ts_, :])

        # layer 1: h^T[ht, tok] = gelu(w1^T x^T), scaled by gate weights
        for ht in range(HT_TILES):
            for c, w in CHUNKS:
                cs = slice(c, c + w)
                h_ps = ps_h.tile([128, w], DT.float32, tag="hps")
                for dt in range(DT_TILES):
                    nc.tensor.matmul(h_ps[:],
                                     w1_sb[:, dt, ht * 128:(ht + 1) * 128],
                                     xg_sb[:, dt, cs],
                                     start=(dt == 0),
                                     stop=(dt == DT_TILES - 1))
                nc.scalar.activation(h_sb[:, ht, cs], h_ps[:], AF.Gelu)
                nc.vector.tensor_mul(h_sb[:, ht, cs], h_sb[:, ht, cs],
                                     wb[:, cs])
        # layer 2: y^T[dt, tok] = w2^T h^T
        for dt in range(DT_TILES):
            for c, w in CHUNKS:
                cs = slice(c, c + w)
                y_ps = ps_y.tile([128, w], DT.float32, tag="yps")
                for ht in range(HT_TILES):
                    nc.tensor.matmul(y_ps[:],
                                     w2_sb[:, ht, dt * 128:(dt + 1) * 128],
                                     h_sb[:, ht, cs],
                                     start=(ht == 0),
                                     stop=(ht == HT_TILES - 1))
                nc.vector.tensor_copy(y_sb[:, dt, cs], y_ps[:])
            # stream this d-tile out while later tiles compute
            nc.sync.dma_start(out=yg_r[:, dt, :], in_=y_sb[:, dt, :])

    nc.compile()
    return nc


def _get_gate_nc():
    global _GATE_NC
    if _GATE_NC is None:
        _GATE_NC = build_gate_nc()
    return _GATE_NC


def _get_ffn_nc():
    global _FFN_NC
    if _FFN_NC is None:
        _FFN_NC = build_ffn_nc()
    return _FFN_NC


def run_gate(xt_full, gate_w, gate_b):
    gb_b = np.ascontiguousarray(np.broadcast_to(gate_b, (128, E))).astype(np.float32)
    gw_pre = np.ascontiguousarray(
        gate_w.reshape(DT_TILES, 128, E).transpose(1, 0, 2))
    ident = np.eye(128, dtype=np.float32)
    in_maps = [{
        "xt": np.ascontiguousarray(xt_full[:, c * TPC:(c + 1) * TPC]),
        "gw": gw_pre,
        "gb": gb_b,
        "idin": ident,
    } for c in range(NCORES)]
    res = run_bass_kernel_spmd(_get_gate_nc(), in_maps,
                               core_ids=list(range(NCORES)))
    wtf = np.concatenate([res.results[c]["wtf"] for c in range(NCORES)],
                         axis=1)                       # [E, NTOK]
    s1 = float(sum(res.results[c]["stats"][0, 0] for c in range(NCORES)))
    s2 = float(sum(res.results[c]["stats"][0, 1] for c in range(NCORES)))
    return wtf, s1, s2


def run_ffn_round(xbT, wtf, idx_lists, w1b, w2b):
    """One expert-parallel round over up to CAP tokens per expert."""
    in_maps = []
    for e in range(NCORES):
        idx = idx_lists[e]
        n = len(idx)
        pad_idx = np.zeros(CAP, dtype=np.int64)
        pad_idx[:n] = idx
        xg = np.ascontiguousarray(xbT[:, pad_idx])     # [D, CAP] bf16
        wg = np.zeros(CAP, dtype=np.float32)
        wg[:n] = wtf[e, idx]
        wgb = np.ascontiguousarray(
            np.broadcast_to(wg.astype(ml_dtypes.bfloat16), (128, CAP)))
        in_maps.append({
            "xg": xg,
            "wgb": wgb,
            "w1": w1b[e],
            "w2": w2b[e],
        })
    res = run_bass_kernel_spmd(_get_ffn_nc(), in_maps,
                               core_ids=list(range(NCORES)))
    return [res.results[e]["yg"] for e in range(NCORES)]


def kernel(x, gate_w, gate_b, w1, w2):
    x = np.asarray(x, dtype=np.float32)
    gate_w = np.asarray(gate_w, dtype=np.float32)
    gate_b = np.asarray(gate_b, dtype=np.float32)
    assert x.shape == (B, S, D), x.shape

    x_flat = x.reshape(NTOK, D)
    xt_full = np.ascontiguousarray(x_flat.T)                      # [D, NTOK] f32
    xbT = xt_full.astype(ml_dtypes.bfloat16)                      # [D, NTOK] bf16
    w1b = np.asarray(w1).astype(ml_dtypes.bfloat16)
    w2b = np.asarray(w2).astype(ml_dtypes.bfloat16)

    # ---- launch A: gate ----
    wtf, s1, s2 = run_gate(xt_full, gate_w, gate_b)

    # ---- host dispatch + launch B rounds + unshard (scatter-add) ----
    full_lists = [np.nonzero(wtf[e])[0] for e in range(E)]

    out_t = np.zeros((NTOK, D), dtype=np.float32)
    offset = 0
    while True:
        round_lists = [fl[offset:offset + CAP] for fl in full_lists]
        if all(len(r) == 0 for r in round_lists):
            break
        ygs = run_ffn_round(xbT, wtf, round_lists, w1b, w2b)
        for e in range(E):
            idx = round_lists[e]
            if len(idx):
                out_t[idx] += ygs[e][:, :len(idx)].T
        offset += CAP

    out = out_t.reshape(B, S, D)

    t = 1.0 / E
    aux = AUX_COEFF * (t * np.log(t) - t * (s1 - E * s2) / (NTOK * E))
    return out, np.float32(aux)
